# revision 35
# baseline (speedup 1.0000x reference)
"""Multi-head attention kernel for Trainium2 (Bass/Tile), 8 NeuronCores.

Problem: nn_MultiHeadAttention
  x [8, 1024, 1024] f32, w_qkv [1024, 3072], b_qkv [3072],
  w_proj [1024, 1024], b_proj [1024]  ->  out [8, 1024, 1024]

  qkv = x @ w_qkv + b_qkv ; split (h, d, 3) interleaved on last dim
  score = q k^T per (b, h);  att = softmax(score, -1) / sqrt(1024)
  out = (att @ v) reshaped @ w_proj + b_proj

Sharding: data-parallel over batch. Each of the 8 cores runs the full
MHA for one batch element; no collectives. Host pre-transposes x and
pre-splits w_qkv so the device program is pure matmul + softmax.

Device-side math per core (all layouts chosen so no on-device transpose
is ever needed):
  qT = (x wq)^T  [(h,d), tok]   lhsT=wq tile, rhs=x^T tile
  kT = (x wk)^T  [(h,d), tok]
  v  = x wv      [tok, (h,d)]   + ones-column per head -> v_aug
  per head: S^T[k,q] = kT.T-slice matmul; E = exp(S^T)
            O'^T[0:64,q] ; O'^T[64,q]=sum_k E  via v_aug ones column
            attoutT = O'[0:64] * (scale / O'[64]) (bcast by PE outer-product)
  out = attoutT.T @ wp + bp   (bias via ones outer-product matmul)
"""

import os

os.environ.setdefault("MYCRO_LOCAL_CACHE", "1")

import numpy as np

import concourse.bass as bass
import concourse.tile as tile
from concourse import bacc, mybir

P = 128
DH = 64  # head dim
F32 = mybir.dt.float32
F32R = mybir.dt.float32r
BF16 = mybir.dt.bfloat16
# matmul-operand dtype: float32r streams at full PE rate (4x fp32);
# values are fp32 bit-patterns rounded by the producing engine.
# The attention-probability path (E = exp(S), v) is bf16: probabilities
# are in [0,1] and v feeds only the softmax average, so 8 mantissa bits
# are plenty, and it halves their SBUF footprint.
MM = F32R

# full-problem constants
B_FULL = 8
TOK_FULL = 1024
D_FULL = 1024
H_FULL = 16
ATT_SCALE_FULL = 1.0 / 32.0  # 1/sqrt(1024), applied after softmax
N_CORES = 8


def _chunks(total, step=512):
    return [(s, min(step, total - s)) for s in range(0, total, step)]


def build(nc, TOK, D, H, att_scale):
    """Emit the one-core MHA program (one batch element).

    DRAM inputs (host pre-laid-out):
      x        [P, KT*TOK]   [p, kt, t] = x[t, kt*P + p]   (x^T, kt-tiled)
      wq/wk/wv/wp [P, KT*D]  [p, kt, n] = w[kt*P + p, n]
      bq/bk    [P, NPAIR]    [p, m] = b[m*P + p]
      bv/bp    [1, D]
    Output: out [TOK, D]

    Structure: V phase up front, then per head pair the attention loop
    with the NEXT pair's Q/K matmul groups interleaved into its k-block
    slots, so the PE never idles on the exp (ACT) dependency chain and
    the HAM clock gate stays at full rate.  PSUM budget (8 banks):
    scores 2 x [128,512] = 2, att@v accumulators 2 x [65,TOK] = 4,
    interleaved Q/K group 2 x [128,512] = 2.
    """
    assert D == H * DH and D % P == 0 and TOK % P == 0 and H % 2 == 0
    KT = D // P       # contraction tiles over the model dim
    MT = TOK // P     # token tiles
    NPAIR = H // 2    # head pairs (== D // P)
    VW = H * (DH + 1)  # v_aug row width: per head [v | 1]
    EXP = mybir.ActivationFunctionType.Exp

    x_d = nc.dram_tensor("x", [P, KT * TOK], MM, kind="ExternalInput")
    xb_d = nc.dram_tensor("xb", [P, KT * TOK], BF16, kind="ExternalInput")
    w_d = {}
    for nm in ("wq", "wk"):
        w_d[nm] = nc.dram_tensor(nm, [P, KT * D], MM, kind="ExternalInput")
    for nm in ("wv", "wp"):
        w_d[nm] = nc.dram_tensor(nm, [P, KT * D], BF16, kind="ExternalInput")
    bq_d = nc.dram_tensor("bq", [P, NPAIR], F32, kind="ExternalInput")
    bk_d = nc.dram_tensor("bk", [P, NPAIR], F32, kind="ExternalInput")
    bv_d = nc.dram_tensor("bv", [1, D], BF16, kind="ExternalInput")
    bp_d = nc.dram_tensor("bp", [1, D], BF16, kind="ExternalInput")
    out_d = nc.dram_tensor("out", [TOK, D], F32, kind="ExternalOutput")

    QH = 512 if TOK >= 512 else TOK  # q-stripe width (PSUM bank = 512 f32)

    with tile.TileContext(nc) as tc:
        with (
            tc.tile_pool(name="sing", bufs=1) as sing,
            tc.tile_pool(name="psS", bufs=2, space="PSUM") as psS,
            tc.tile_pool(name="psQK", bufs=2, space="PSUM") as psQK,
            tc.tile_pool(name="psB", bufs=4, space="PSUM") as psB,
            tc.tile_pool(name="ebuf", bufs=8) as ebuf,
            tc.tile_pool(name="rbuf", bufs=2) as rbuf,
            tc.tile_pool(name="bcp", bufs=2) as bcp,
            tc.tile_pool(name="outp", bufs=2) as outp,
            tc.tile_pool(name="wqkp", bufs=3) as wqkp,
        ):
            # ---------------- persistent SBUF ----------------
            # v_aug denominator columns carry 1/att_scale so the softmax
            # denominator comes out pre-divided by att_scale: recip of it
            # directly yields att_scale / sum(exp)
            vones_sb = sing.tile([P, MT * H], F32, tag="vones")
            nc.vector.memset(vones_sb, 1.0 / att_scale)

            bq_sb = sing.tile([P, NPAIR], F32, tag="bq")
            nc.sync.dma_start(out=bq_sb, in_=bq_d[:, :])
            bk_sb = sing.tile([P, NPAIR], F32, tag="bk")
            nc.sync.dma_start(out=bk_sb, in_=bk_d[:, :])
            # biases enter via DVE adds fused into the existing PSUM
            # drains (no PE outer-product matmuls): broadcast them across
            # partitions once on the idle GPSIMD engine
            bv_sb = sing.tile([1, D], BF16, tag="bv")
            nc.sync.dma_start(out=bv_sb, in_=bv_d[:, :])
            bp_sb = sing.tile([1, D], BF16, tag="bp")
            nc.sync.dma_start(out=bp_sb, in_=bp_d[:, :])
            bv_bc = sing.tile([P, D], BF16, tag="bvbc")
            nc.gpsimd.partition_broadcast(bv_bc[:, :], bv_sb[0:1, :], channels=P)
            bp_bc = sing.tile([P, D], BF16, tag="bpbc")
            nc.gpsimd.partition_broadcast(bp_bc[:, :], bp_sb[0:1, :], channels=P)

            v_sb = sing.tile([P, MT, VW], BF16, tag="v")     # v_aug
            nc.vector.tensor_copy(
                out=v_sb[:, :, :]
                .rearrange("p m (h e) -> p m h e", e=DH + 1)[:, :, :, DH],
                in_=vones_sb[:, :].rearrange("p (m h) -> p m h", h=H),
            )
            qT_sb = sing.tile([P, NPAIR, TOK], MM, tag="qT")
            kT_sb = sing.tile([P, NPAIR, TOK], MM, tag="kT")
            ao_sb = sing.tile([P, NPAIR, TOK], BF16, tag="ao")  # attout^T

            with tc.tile_pool(name="xp", bufs=1) as xp:
                x_sb = xp.tile([P, KT * TOK], MM, tag="x")
                x3 = x_sb[:, :].rearrange("p (kt t) -> p kt t", t=TOK)
                x_d3 = x_d[:, :].rearrange("p (kt t) -> p kt t", t=TOK)
                w_d3 = {
                    nm: w_d[nm][:, :].rearrange("p (kt n) -> p kt n", n=D)
                    for nm in w_d
                }

                # per-pair Q/K weight tiles, DMA'd one pair ahead
                w_tiles = {}

                def fetch_qk_pair(pg):
                    for wname in ("wq", "wk"):
                        wt = wqkp.tile([P, KT, P], MM, tag="w" + str(pg % 2))
                        nc.sync.dma_start(
                            out=wt,
                            in_=w_d3[wname][:, :, pg * P : (pg + 1) * P],
                        )
                        w_tiles[(wname, pg)] = wt

                # Q/K matmul-group stream, interleaved into attention slots
                qk_state = {"groups": [], "gi": 0, "kt": 0, "ps": None}

                def qk_add_pair(pg):
                    for wname, dst_sb, b_sb in (
                        ("wq", qT_sb, bq_sb),
                        ("wk", kT_sb, bk_sb),
                    ):
                        for c0, cw in _chunks(TOK, QH):
                            qk_state["groups"].append(
                                (wname, pg, c0, cw, dst_sb, b_sb)
                            )

                def qk_emit(n):
                    for _ in range(n):
                        if qk_state["gi"] >= len(qk_state["groups"]):
                            return
                        wname, pg, c0, cw, dst_sb, b_sb = qk_state["groups"][
                            qk_state["gi"]
                        ]
                        kt = qk_state["kt"]
                        if kt == 0:
                            ps_qk = psQK.tile([P, QH], F32, tag="psQK")
                            qk_state["ps"] = ps_qk
                        nc.tensor.matmul(
                            qk_state["ps"][:, 0:cw],
                            lhsT=w_tiles[(wname, pg)][:, kt, :],
                            rhs=x3[:, kt, c0 : c0 + cw],
                            start=(kt == 0),
                            stop=(kt == KT - 1),
                        )
                        if kt == KT - 1:
                            nc.vector.tensor_scalar_add(
                                out=dst_sb[:, pg, c0 : c0 + cw],
                                in0=qk_state["ps"][:, 0:cw],
                                scalar1=b_sb[:, pg : pg + 1],
                            )
                            qk_state["kt"] = 0
                            qk_state["gi"] += 1
                        else:
                            qk_state["kt"] = kt + 1

                # ---------------- DMA prologue ----------------
                # The V phase runs on bf16 copies of x and wv (half the
                # bytes on the DMA-bound critical path).  Dependency
                # tracking is tile-granular, so each token-quarter of xb
                # and column-chunk of wv gets its OWN tile: the first V
                # matmul then waits only on the two small DMAs it reads.
                with tc.tile_pool(name="wvp", bufs=1) as wvp:
                    xb_d3 = xb_d[:, :].rearrange("p (kt t) -> p kt t", t=TOK)
                    vchunks = _chunks(D, 512)
                    xq_w = max(P, TOK // 4)
                    mpq = xq_w // P  # mt-blocks per xb quarter tile
                    wv_tiles = []
                    xb_tiles = []
                    for ci, (c0, cw) in enumerate(vchunks):
                        wvt = wvp.tile([P, KT, cw], BF16, tag="wv" + str(ci))
                        wv_tiles.append(wvt)
                    for qq in range(TOK // xq_w):
                        xbt = wvp.tile([P, KT, xq_w], BF16, tag="xq" + str(qq))
                        xb_tiles.append(xbt)
                    nc.sync.dma_start(
                        out=wv_tiles[0],
                        in_=w_d3["wv"][:, :, 0 : vchunks[0][1]],
                    )
                    nc.sync.dma_start(
                        out=xb_tiles[0], in_=xb_d3[:, :, 0:xq_w]
                    )
                    for ci, (c0, cw) in list(enumerate(vchunks))[1:]:
                        nc.sync.dma_start(
                            out=wv_tiles[ci], in_=w_d3["wv"][:, :, c0 : c0 + cw]
                        )
                    for qq in range(1, TOK // xq_w):
                        nc.sync.dma_start(
                            out=xb_tiles[qq],
                            in_=xb_d3[:, :, qq * xq_w : (qq + 1) * xq_w],
                        )
                    # full-precision x (for Q/K) streams in behind the bf16
                    # copies; it is only needed once pair-0 Q/K starts
                    nc.sync.dma_start(
                        out=x_sb[:, :], in_=x_d[:, :]
                    )
                    fetch_qk_pair(0)

                    # ------------ V phase: v = x wv + bv (natural) ----
                    # mt-major so compute follows the token-quarter DMAs
                    for mt in range(MT):
                        for ci, (c0, cw) in enumerate(vchunks):
                            ps_v = psS.tile([P, QH], F32, tag="psS")
                            for kt in range(KT):
                                nc.tensor.matmul(
                                    ps_v[:, 0:cw],
                                    lhsT=xb_tiles[mt // mpq][
                                        :, kt, (mt % mpq) * P : (mt % mpq + 1) * P
                                    ],
                                    rhs=wv_tiles[ci][:, kt, 0:cw],
                                    start=(kt == 0),
                                    stop=(kt == KT - 1),
                                )
                            # scatter heads into v_aug (65-stride), adding
                            # bv during the drain
                            nh = cw // DH
                            h0 = c0 // DH
                            nc.vector.tensor_add(
                                out=v_sb[:, mt, :]
                                .rearrange("p (h e) -> p h e", e=DH + 1)[
                                    :, h0 : h0 + nh, 0:DH
                                ],
                                in0=ps_v[:, 0:cw].rearrange(
                                    "p (h d) -> p h d", d=DH
                                ),
                                in1=bv_bc[:, c0 : c0 + cw].rearrange(
                                    "p (h d) -> p h d", d=DH
                                ),
                            )

                # ---------------- pair-0 Q/K, up front ----------------
                qk_add_pair(0)
                qk_emit(4 * KT)

                # -------- attention + interleaved next-pair Q/K -------
                def emit_scores_half(p, kb, q0, qw):
                    """Both heads' scores for one q stripe; the two matmuls
                    land in different PE row groups (base partitions 0/64)
                    and run concurrently."""
                    pss = []
                    for base in (0, DH):
                        ps = psS.tile([P, QH], F32, tag="psS")
                        nc.tensor.matmul(
                            ps[:, 0:qw],
                            lhsT=kT_sb[
                                base : base + DH, p, kb * P : (kb + 1) * P
                            ],
                            rhs=qT_sb[base : base + DH, p, q0 : q0 + qw],
                            start=True,
                            stop=True,
                        )
                        pss.append(ps)
                    ets = []
                    for ps in pss:
                        et = ebuf.tile([P, QH], BF16, tag="E")
                        nc.scalar.activation(
                            out=et[:, 0:qw], in_=ps[:, 0:qw], func=EXP
                        )
                        ets.append(et)
                    return ets

                def attn_pair(p, filler=None):
                    if p + 1 < NPAIR:
                        fetch_qk_pair(p + 1)
                        qk_add_pair(p + 1)
                    # one accumulator tile per (head, q-stripe): each is a
                    # single PSUM bank, so a stripe's bank frees as soon as
                    # ITS normalize mul has read it -- the next pair's
                    # att@v no longer waits for the whole pair to drain
                    qhalves = _chunks(TOK, QH)
                    acc = {}
                    for qi in range(len(qhalves)):
                        for hoff in (0, 1):
                            ps_acc = psB.tile([DH + 1, QH], F32, tag="psB")
                            acc[(hoff, qi)] = ps_acc
                    nrm = {}
                    ecur = [emit_scores_half(p, 0, q0, qw) for q0, qw in qhalves]
                    for kb in range(MT):
                        for qi, (q0, qw) in enumerate(qhalves):
                            enext = None
                            if kb + 1 < MT:
                                enext = emit_scores_half(p, kb + 1, q0, qw)
                            for hoff in (0, 1):
                                ps_out = acc[(hoff, qi)]
                                hh = 2 * p + hoff
                                nc.tensor.matmul(
                                    ps_out[:, 0:qw],
                                    lhsT=v_sb[
                                        :,
                                        kb,
                                        hh * (DH + 1) : (hh + 1) * (DH + 1),
                                    ],
                                    rhs=ecur[qi][hoff][:, 0:qw],
                                    start=(kb == 0),
                                    stop=(kb == MT - 1),
                                    skip_group_check=True,
                                )
                            qk_emit(2)
                            if filler is not None:
                                filler(2)
                            if kb == MT - 1:
                                # normalize this q stripe as soon as its
                                # accumulation closes:
                                #   attout^T = O'[0:DH] * recip(O'[DH])
                                # (att_scale is folded into the v_aug ones
                                # column).  reciprocal_approx_fast silently
                                # corrupts on HW when its input AP starts at
                                # partition 64, so stage the denominator row
                                # at partition 0 first (row 0 of the bc
                                # tile, which the broadcast then overwrites
                                # -- its RAW dep on r guarantees the recip
                                # consumed it first).
                                if qi == 0:
                                    r_e = rbuf.tile([1, TOK], F32, tag="R")
                                    r_o = rbuf.tile([1, TOK], F32, tag="R")
                                    bc_e = bcp.tile([DH, TOK], F32, tag="BC")
                                    bc_o = bcp.tile([DH, TOK], F32, tag="BC")
                                    nrm[0] = (r_e, bc_e)
                                    nrm[1] = (r_o, bc_o)
                                for ri in (0, 1):
                                    r_sb, bc_sb = nrm[ri]
                                    ps_o = acc[(ri, qi)]
                                    nc.vector.tensor_copy(
                                        out=bc_sb[0:1, q0 : q0 + qw],
                                        in_=ps_o[DH : DH + 1, 0:qw],
                                    )
                                    nc.vector.reciprocal_approx_fast(
                                        out=r_sb[0:1, q0 : q0 + qw],
                                        in_=bc_sb[0:1, q0 : q0 + qw],
                                    )
                                    nc.gpsimd.partition_broadcast(
                                        bc_sb[:, q0 : q0 + qw],
                                        r_sb[0:1, q0 : q0 + qw],
                                        channels=DH,
                                    )
                                    nc.vector.tensor_mul(
                                        out=ao_sb[
                                            ri * DH : (ri + 1) * DH,
                                            p,
                                            q0 : q0 + qw,
                                        ],
                                        in0=ps_o[0:DH, 0:qw],
                                        in1=bc_sb[:, q0 : q0 + qw],
                                    )
                            if enext is not None:
                                ecur[qi] = enext

                for p in range(NPAIR - 1):
                    attn_pair(p)

            # x freed: the wp prefetch DMAs (arena-aliased with x) run
            # during the last pair's attention, and the last pair's slots
            # (which have no Q/K work left) are filled with the first
            # projection groups' partial contractions (kt <= KT-2; the
            # pair's own ao only enters at kt = KT-1).  Projection PSUM
            # comes from psQK -- free once the Q/K stream is exhausted --
            # so no proj matmul ever waits on the attention accumulators.
            with tc.tile_pool(name="wpp", bufs=2) as wpp:
                w_dp = w_d["wp"][:, :].rearrange("p (kt n) -> p kt n", n=D)
                pj_groups = []
                for c0, cw in _chunks(D, 512):
                    wp_sb = wpp.tile([P, KT, 512], BF16, tag="wp")
                    nc.sync.dma_start(
                        out=wp_sb[:, :, 0:cw], in_=w_dp[:, :, c0 : c0 + cw]
                    )
                    for mt in range(MT):
                        pj_groups.append((c0, cw, wp_sb, mt))

                pj = {"gi": 0, "kt": 0, "ps": None}

                def pj_emit(n, during_attn=False):
                    for _ in range(n):
                        gi = pj["gi"]
                        if gi >= len(pj_groups):
                            return
                        if during_attn and (gi >= 2 or pj["kt"] >= KT - 1):
                            return
                        c0, cw, wp_sb, mt = pj_groups[gi]
                        kt = pj["kt"]
                        if kt == 0:
                            ps_p = psQK.tile([P, 512], F32, tag="psQK")
                            pj["ps"] = ps_p
                        nc.tensor.matmul(
                            pj["ps"][:, 0:cw],
                            lhsT=ao_sb[:, kt, mt * P : (mt + 1) * P],
                            rhs=wp_sb[:, kt, 0:cw],
                            start=(kt == 0),
                            stop=(kt == KT - 1),
                        )
                        if kt == KT - 1:
                            o_sb = outp.tile([P, 512], F32, tag="o")
                            nc.vector.tensor_add(
                                out=o_sb[:, 0:cw],
                                in0=pj["ps"][:, 0:cw],
                                in1=bp_bc[:, c0 : c0 + cw],
                            )
                            nc.sync.dma_start(
                                out=out_d[mt * P : (mt + 1) * P, c0 : c0 + cw],
                                in_=o_sb[:, 0:cw],
                            )
                            pj["kt"] = 0
                            pj["gi"] += 1
                        else:
                            pj["kt"] = kt + 1

                attn_pair(NPAIR - 1, filler=lambda n: pj_emit(n, True))
                pj_emit(len(pj_groups) * (KT + 1))

    return nc


# ---------------------------------------------------------------------------
# host-side layout prep
# ---------------------------------------------------------------------------

def _round_f32r(x):
    """RNE to f32r's 11-explicit-mantissa-bit grid (matches HW rounding)."""
    u = np.ascontiguousarray(x, np.float32).view(np.uint32)
    u = ((u + np.uint32(1 << 11)) >> 12) << 12
    return u.view(np.float32)


def host_prep_shared(w_qkv, b_qkv, w_proj, b_proj, D, H):
    """Split/retile the weights once for all cores."""
    KT = D // P
    NPAIR = H // 2

    def tile_w(w):  # [D, N] -> [P, KT*N]
        N = w.shape[1]
        return _round_f32r(
            w.reshape(KT, P, N).transpose(1, 0, 2).reshape(P, KT * N)
        )

    def tile_w_bf16(w):
        import ml_dtypes

        N = w.shape[1]
        return np.ascontiguousarray(
            w.reshape(KT, P, N).transpose(1, 0, 2).reshape(P, KT * N)
        ).astype(ml_dtypes.bfloat16)

    wq3 = w_qkv.reshape(D, H, DH, 3)
    out = {
        "wq": tile_w(np.ascontiguousarray(wq3[:, :, :, 0].reshape(D, D))),
        "wk": tile_w(np.ascontiguousarray(wq3[:, :, :, 1].reshape(D, D))),
        "wv": tile_w_bf16(np.ascontiguousarray(wq3[:, :, :, 2].reshape(D, D))),
        "wp": tile_w_bf16(np.ascontiguousarray(w_proj)),
    }
    b3 = b_qkv.reshape(H, DH, 3)
    bq = np.ascontiguousarray(b3[:, :, 0].reshape(D))
    bk = np.ascontiguousarray(b3[:, :, 1].reshape(D))
    bv = np.ascontiguousarray(b3[:, :, 2].reshape(D))
    out["bq"] = np.ascontiguousarray(bq.reshape(NPAIR, P).T).astype(np.float32)
    out["bk"] = np.ascontiguousarray(bk.reshape(NPAIR, P).T).astype(np.float32)
    import ml_dtypes

    out["bv"] = bv.reshape(1, D).astype(ml_dtypes.bfloat16)
    out["bp"] = np.asarray(b_proj, np.float32).reshape(1, D).astype(
        ml_dtypes.bfloat16
    )
    return out


def host_prep_x(x_b, TOK, D):
    """One batch element [TOK, D] -> x^T tiled [P, KT*TOK]."""
    KT = D // P
    xT = np.ascontiguousarray(np.asarray(x_b, np.float32).T)  # [D, TOK]
    return _round_f32r(
        xT.reshape(KT, P, TOK).transpose(1, 0, 2).reshape(P, KT * TOK)
    )


def host_prep_x_bf16(x_b, TOK, D):
    import ml_dtypes

    KT = D // P
    xT = np.ascontiguousarray(np.asarray(x_b, np.float32).T)
    return (
        xT.reshape(KT, P, TOK)
        .transpose(1, 0, 2)
        .reshape(P, KT * TOK)
        .astype(ml_dtypes.bfloat16)
    )


# ---------------------------------------------------------------------------
# entry point
# ---------------------------------------------------------------------------

_BUILT = {}


def _get_nc(TOK, D, H, att_scale):
    key = (TOK, D, H, att_scale)
    if key not in _BUILT:
        nc = bacc.Bacc(
            "TRN2",
            target_bir_lowering=False,
            debug=False,
            dynamic_dma_scratch_size=512,
        )
        build(nc, TOK, D, H, att_scale)
        nc.compile()
        nc.finalize()
        _BUILT[key] = nc
    return _BUILT[key]


def kernel(x, w_qkv, b_qkv, w_proj, b_proj):
    from concourse.bass_utils import run_bass_kernel_spmd

    x = np.asarray(x, np.float32)
    B, TOK, D = x.shape
    H = H_FULL
    shared = host_prep_shared(
        np.asarray(w_qkv, np.float32),
        np.asarray(b_qkv, np.float32),
        np.asarray(w_proj, np.float32),
        np.asarray(b_proj, np.float32),
        D,
        H,
    )
    in_maps = []
    for b in range(B):
        m = dict(shared)
        m["x"] = host_prep_x(x[b], TOK, D)
        m["xb"] = host_prep_x_bf16(x[b], TOK, D)
        in_maps.append(m)

    nc = _get_nc(TOK, D, H, ATT_SCALE_FULL)
    res = run_bass_kernel_spmd(nc, in_maps, list(range(N_CORES)))
    out = np.stack([res.results[b]["out"] for b in range(B)], axis=0)
    return out.astype(np.float32)



# revision 36
# speedup vs baseline: 1.0228x; 1.0228x over previous
"""Multi-head attention kernel for Trainium2 (Bass/Tile), 8 NeuronCores.

Problem: nn_MultiHeadAttention
  x [8, 1024, 1024] f32, w_qkv [1024, 3072], b_qkv [3072],
  w_proj [1024, 1024], b_proj [1024]  ->  out [8, 1024, 1024]

  qkv = x @ w_qkv + b_qkv ; split (h, d, 3) interleaved on last dim
  score = q k^T per (b, h);  att = softmax(score, -1) / sqrt(1024)
  out = (att @ v) reshaped @ w_proj + b_proj

Sharding: data-parallel over batch. Each of the 8 cores runs the full
MHA for one batch element; no collectives. Host pre-transposes x and
pre-splits w_qkv so the device program is pure matmul + softmax.

Device-side math per core (all layouts chosen so no on-device transpose
is ever needed):
  qT = (x wq)^T  [(h,d), tok]   lhsT=wq tile, rhs=x^T tile
  kT = (x wk)^T  [(h,d), tok]
  v  = x wv      [tok, (h,d)]   + ones-column per head -> v_aug
  per head: S^T[k,q] = kT.T-slice matmul; E = exp(S^T)
            O'^T[0:64,q] ; O'^T[64,q]=sum_k E  via v_aug ones column
            attoutT = O'[0:64] * (scale / O'[64]) (bcast by PE outer-product)
  out = attoutT.T @ wp + bp   (bias via ones outer-product matmul)
"""

import os

os.environ.setdefault("MYCRO_LOCAL_CACHE", "1")

import numpy as np

import concourse.bass as bass
import concourse.tile as tile
from concourse import bacc, mybir

P = 128
DH = 64  # head dim
F32 = mybir.dt.float32
F32R = mybir.dt.float32r
BF16 = mybir.dt.bfloat16
# matmul-operand dtype: float32r streams at full PE rate (4x fp32);
# values are fp32 bit-patterns rounded by the producing engine.
# The attention-probability path (E = exp(S), v) is bf16: probabilities
# are in [0,1] and v feeds only the softmax average, so 8 mantissa bits
# are plenty, and it halves their SBUF footprint.
MM = F32R

# full-problem constants
B_FULL = 8
TOK_FULL = 1024
D_FULL = 1024
H_FULL = 16
ATT_SCALE_FULL = 1.0 / 32.0  # 1/sqrt(1024), applied after softmax
N_CORES = 8


def _chunks(total, step=512):
    return [(s, min(step, total - s)) for s in range(0, total, step)]


def build(nc, TOK, D, H, att_scale):
    """Emit the one-core MHA program (one batch element).

    DRAM inputs (host pre-laid-out):
      x        [P, KT*TOK]   [p, kt, t] = x[t, kt*P + p]   (x^T, kt-tiled)
      wq/wk/wv/wp [P, KT*D]  [p, kt, n] = w[kt*P + p, n]
      bq/bk    [P, NPAIR]    [p, m] = b[m*P + p]
      bv/bp    [1, D]
    Output: out [TOK, D]

    Structure: V phase up front, then per head pair the attention loop
    with the NEXT pair's Q/K matmul groups interleaved into its k-block
    slots, so the PE never idles on the exp (ACT) dependency chain and
    the HAM clock gate stays at full rate.  PSUM budget (8 banks):
    scores 2 x [128,512] = 2, att@v accumulators 2 x [65,TOK] = 4,
    interleaved Q/K group 2 x [128,512] = 2.
    """
    assert D == H * DH and D % P == 0 and TOK % P == 0 and H % 2 == 0
    KT = D // P       # contraction tiles over the model dim
    MT = TOK // P     # token tiles
    NPAIR = H // 2    # head pairs (== D // P)
    VW = H * (DH + 1)  # v_aug row width: per head [v | 1]
    EXP = mybir.ActivationFunctionType.Exp

    x_d = nc.dram_tensor("x", [P, KT * TOK], MM, kind="ExternalInput")
    xb_d = nc.dram_tensor("xb", [P, KT * TOK], BF16, kind="ExternalInput")
    w_d = {}
    for nm in ("wq", "wk"):
        w_d[nm] = nc.dram_tensor(nm, [P, KT * D], MM, kind="ExternalInput")
    for nm in ("wv", "wp"):
        w_d[nm] = nc.dram_tensor(nm, [P, KT * D], BF16, kind="ExternalInput")
    bq_d = nc.dram_tensor("bq", [P, NPAIR], F32, kind="ExternalInput")
    bk_d = nc.dram_tensor("bk", [P, NPAIR], F32, kind="ExternalInput")
    bv_d = nc.dram_tensor("bv", [1, D], BF16, kind="ExternalInput")
    bp_d = nc.dram_tensor("bp", [1, D], BF16, kind="ExternalInput")
    out_d = nc.dram_tensor("out", [TOK, D], F32, kind="ExternalOutput")

    QH = 512 if TOK >= 512 else TOK  # q-stripe width (PSUM bank = 512 f32)

    with tile.TileContext(nc) as tc:
        with (
            tc.tile_pool(name="sing", bufs=1) as sing,
            tc.tile_pool(name="psS", bufs=2, space="PSUM") as psS,
            tc.tile_pool(name="psQK", bufs=2, space="PSUM") as psQK,
            tc.tile_pool(name="psB", bufs=2, space="PSUM") as psB,
            tc.tile_pool(name="ebuf", bufs=8) as ebuf,
            tc.tile_pool(name="rbuf", bufs=2) as rbuf,
            tc.tile_pool(name="bcp", bufs=2) as bcp,
            tc.tile_pool(name="outp", bufs=2) as outp,
            tc.tile_pool(name="wqkp", bufs=3) as wqkp,
        ):
            # ---------------- persistent SBUF ----------------
            # v_aug denominator columns carry 1/att_scale so the softmax
            # denominator comes out pre-divided by att_scale: recip of it
            # directly yields att_scale / sum(exp)
            vones_sb = sing.tile([P, MT * H], F32, tag="vones")
            nc.vector.memset(vones_sb, 1.0 / att_scale)

            bq_sb = sing.tile([P, NPAIR], F32, tag="bq")
            nc.sync.dma_start(out=bq_sb, in_=bq_d[:, :])
            bk_sb = sing.tile([P, NPAIR], F32, tag="bk")
            nc.sync.dma_start(out=bk_sb, in_=bk_d[:, :])
            # biases enter via DVE adds fused into the existing PSUM
            # drains (no PE outer-product matmuls): broadcast them across
            # partitions once on the idle GPSIMD engine
            bv_sb = sing.tile([1, D], BF16, tag="bv")
            nc.sync.dma_start(out=bv_sb, in_=bv_d[:, :])
            bp_sb = sing.tile([1, D], BF16, tag="bp")
            nc.sync.dma_start(out=bp_sb, in_=bp_d[:, :])
            bv_bc = sing.tile([P, D], BF16, tag="bvbc")
            nc.gpsimd.partition_broadcast(bv_bc[:, :], bv_sb[0:1, :], channels=P)
            bp_bc = sing.tile([P, D], BF16, tag="bpbc")
            nc.gpsimd.partition_broadcast(bp_bc[:, :], bp_sb[0:1, :], channels=P)

            v_sb = sing.tile([P, MT, VW], BF16, tag="v")     # v_aug
            nc.vector.tensor_copy(
                out=v_sb[:, :, :]
                .rearrange("p m (h e) -> p m h e", e=DH + 1)[:, :, :, DH],
                in_=vones_sb[:, :].rearrange("p (m h) -> p m h", h=H),
            )
            qT_sb = sing.tile([P, NPAIR, TOK], MM, tag="qT")
            kT_sb = sing.tile([P, NPAIR, TOK], MM, tag="kT")
            ao_sb = sing.tile([P, NPAIR, TOK], BF16, tag="ao")  # attout^T

            with tc.tile_pool(name="xp", bufs=1) as xp:
                x_sb = xp.tile([P, KT * TOK], MM, tag="x")
                x3 = x_sb[:, :].rearrange("p (kt t) -> p kt t", t=TOK)
                x_d3 = x_d[:, :].rearrange("p (kt t) -> p kt t", t=TOK)
                w_d3 = {
                    nm: w_d[nm][:, :].rearrange("p (kt n) -> p kt n", n=D)
                    for nm in w_d
                }

                # per-pair Q/K weight tiles, DMA'd one pair ahead
                w_tiles = {}

                def fetch_qk_pair(pg):
                    for wname in ("wq", "wk"):
                        wt = wqkp.tile([P, KT, P], MM, tag="w" + str(pg % 2))
                        nc.sync.dma_start(
                            out=wt,
                            in_=w_d3[wname][:, :, pg * P : (pg + 1) * P],
                        )
                        w_tiles[(wname, pg)] = wt

                # Q/K matmul-group stream, interleaved into attention slots
                qk_state = {"groups": [], "gi": 0, "kt": 0, "ps": None}

                def qk_add_pair(pg):
                    for wname, dst_sb, b_sb in (
                        ("wq", qT_sb, bq_sb),
                        ("wk", kT_sb, bk_sb),
                    ):
                        for c0, cw in _chunks(TOK, QH):
                            qk_state["groups"].append(
                                (wname, pg, c0, cw, dst_sb, b_sb)
                            )

                def qk_emit(n):
                    for _ in range(n):
                        if qk_state["gi"] >= len(qk_state["groups"]):
                            return
                        wname, pg, c0, cw, dst_sb, b_sb = qk_state["groups"][
                            qk_state["gi"]
                        ]
                        kt = qk_state["kt"]
                        if kt == 0:
                            ps_qk = psQK.tile([P, QH], F32, tag="psQK")
                            qk_state["ps"] = ps_qk
                        nc.tensor.matmul(
                            qk_state["ps"][:, 0:cw],
                            lhsT=w_tiles[(wname, pg)][:, kt, :],
                            rhs=x3[:, kt, c0 : c0 + cw],
                            start=(kt == 0),
                            stop=(kt == KT - 1),
                        )
                        if kt == KT - 1:
                            nc.vector.tensor_scalar_add(
                                out=dst_sb[:, pg, c0 : c0 + cw],
                                in0=qk_state["ps"][:, 0:cw],
                                scalar1=b_sb[:, pg : pg + 1],
                            )
                            qk_state["kt"] = 0
                            qk_state["gi"] += 1
                        else:
                            qk_state["kt"] = kt + 1

                # ---------------- DMA prologue ----------------
                # The V phase runs on bf16 copies of x and wv (half the
                # bytes on the DMA-bound critical path).  Dependency
                # tracking is tile-granular, so each token-quarter of xb
                # and column-chunk of wv gets its OWN tile: the first V
                # matmul then waits only on the two small DMAs it reads.
                with tc.tile_pool(name="wvp", bufs=1) as wvp:
                    xb_d3 = xb_d[:, :].rearrange("p (kt t) -> p kt t", t=TOK)
                    vchunks = _chunks(D, 512)
                    xq_w = max(P, TOK // 4)
                    mpq = xq_w // P  # mt-blocks per xb quarter tile
                    wv_tiles = []
                    xb_tiles = []
                    for ci, (c0, cw) in enumerate(vchunks):
                        wvt = wvp.tile([P, KT, cw], BF16, tag="wv" + str(ci))
                        wv_tiles.append(wvt)
                    for qq in range(TOK // xq_w):
                        xbt = wvp.tile([P, KT, xq_w], BF16, tag="xq" + str(qq))
                        xb_tiles.append(xbt)
                    nc.sync.dma_start(
                        out=wv_tiles[0],
                        in_=w_d3["wv"][:, :, 0 : vchunks[0][1]],
                    )
                    nc.sync.dma_start(
                        out=xb_tiles[0], in_=xb_d3[:, :, 0:xq_w]
                    )
                    for ci, (c0, cw) in list(enumerate(vchunks))[1:]:
                        nc.sync.dma_start(
                            out=wv_tiles[ci], in_=w_d3["wv"][:, :, c0 : c0 + cw]
                        )
                    for qq in range(1, TOK // xq_w):
                        nc.sync.dma_start(
                            out=xb_tiles[qq],
                            in_=xb_d3[:, :, qq * xq_w : (qq + 1) * xq_w],
                        )
                    # full-precision x (for Q/K) streams in behind the bf16
                    # copies; it is only needed once pair-0 Q/K starts
                    nc.sync.dma_start(
                        out=x_sb[:, :], in_=x_d[:, :]
                    )
                    fetch_qk_pair(0)

                    # ------------ V phase: v = x wv + bv (natural) ----
                    # mt-major so compute follows the token-quarter DMAs
                    for mt in range(MT):
                        for ci, (c0, cw) in enumerate(vchunks):
                            ps_v = psS.tile([P, QH], F32, tag="psS")
                            for kt in range(KT):
                                nc.tensor.matmul(
                                    ps_v[:, 0:cw],
                                    lhsT=xb_tiles[mt // mpq][
                                        :, kt, (mt % mpq) * P : (mt % mpq + 1) * P
                                    ],
                                    rhs=wv_tiles[ci][:, kt, 0:cw],
                                    start=(kt == 0),
                                    stop=(kt == KT - 1),
                                )
                            # scatter heads into v_aug (65-stride), adding
                            # bv during the drain
                            nh = cw // DH
                            h0 = c0 // DH
                            nc.vector.tensor_add(
                                out=v_sb[:, mt, :]
                                .rearrange("p (h e) -> p h e", e=DH + 1)[
                                    :, h0 : h0 + nh, 0:DH
                                ],
                                in0=ps_v[:, 0:cw].rearrange(
                                    "p (h d) -> p h d", d=DH
                                ),
                                in1=bv_bc[:, c0 : c0 + cw].rearrange(
                                    "p (h d) -> p h d", d=DH
                                ),
                            )

                # ---------------- pair-0 Q/K, up front ----------------
                qk_add_pair(0)
                qk_emit(4 * KT)

                # -------- attention + interleaved next-pair Q/K -------
                def emit_scores_half(p, kb, q0, qw):
                    """Both heads' scores for one q stripe; the two matmuls
                    land in different PE row groups (base partitions 0/64)
                    and run concurrently."""
                    pss = []
                    for base in (0, DH):
                        ps = psS.tile([P, QH], F32, tag="psS")
                        nc.tensor.matmul(
                            ps[:, 0:qw],
                            lhsT=kT_sb[
                                base : base + DH, p, kb * P : (kb + 1) * P
                            ],
                            rhs=qT_sb[base : base + DH, p, q0 : q0 + qw],
                            start=True,
                            stop=True,
                        )
                        pss.append(ps)
                    ets = []
                    for ps in pss:
                        et = ebuf.tile([P, QH], BF16, tag="E")
                        nc.scalar.activation(
                            out=et[:, 0:qw], in_=ps[:, 0:qw], func=EXP
                        )
                        ets.append(et)
                    return ets

                def attn_pair(p, filler=None):
                    if p + 1 < NPAIR:
                        fetch_qk_pair(p + 1)
                        qk_add_pair(p + 1)
                    ps_oe = psB.tile([DH + 1, TOK], F32, tag="psB")
                    ps_oo = psB.tile([DH + 1, TOK], F32, tag="psB")
                    nrm = {}

                    qhalves = _chunks(TOK, QH)
                    ecur = [emit_scores_half(p, 0, q0, qw) for q0, qw in qhalves]
                    for kb in range(MT):
                        for qi, (q0, qw) in enumerate(qhalves):
                            enext = None
                            if kb + 1 < MT:
                                enext = emit_scores_half(p, kb + 1, q0, qw)
                            for hoff, ps_out in ((0, ps_oe), (1, ps_oo)):
                                hh = 2 * p + hoff
                                nc.tensor.matmul(
                                    ps_out[:, q0 : q0 + qw],
                                    lhsT=v_sb[
                                        :,
                                        kb,
                                        hh * (DH + 1) : (hh + 1) * (DH + 1),
                                    ],
                                    rhs=ecur[qi][hoff][:, 0:qw],
                                    start=(kb == 0),
                                    stop=(kb == MT - 1),
                                    skip_group_check=True,
                                )
                            qk_emit(2)
                            if filler is not None:
                                filler(2)
                            if kb == MT - 1:
                                # normalize this q stripe as soon as its
                                # accumulation closes:
                                #   attout^T = O'[0:DH] * recip(O'[DH])
                                # (att_scale is folded into the v_aug ones
                                # column).  reciprocal_approx_fast silently
                                # corrupts on HW when its input AP starts at
                                # partition 64, so stage the denominator row
                                # at partition 0 first (row 0 of the bc
                                # tile, which the broadcast then overwrites
                                # -- its RAW dep on r guarantees the recip
                                # consumed it first).
                                if qi == 0:
                                    r_e = rbuf.tile([1, TOK], F32, tag="R")
                                    r_o = rbuf.tile([1, TOK], F32, tag="R")
                                    bc_e = bcp.tile([DH, TOK], F32, tag="BC")
                                    bc_o = bcp.tile([DH, TOK], F32, tag="BC")
                                    nrm[0] = (ps_oe, r_e, bc_e)
                                    nrm[1] = (ps_oo, r_o, bc_o)
                                for ri in (0, 1):
                                    ps_o, r_sb, bc_sb = nrm[ri]
                                    nc.vector.tensor_copy(
                                        out=bc_sb[0:1, q0 : q0 + qw],
                                        in_=ps_o[DH : DH + 1, q0 : q0 + qw],
                                    )
                                    nc.vector.reciprocal_approx_fast(
                                        out=r_sb[0:1, q0 : q0 + qw],
                                        in_=bc_sb[0:1, q0 : q0 + qw],
                                    )
                                    nc.gpsimd.partition_broadcast(
                                        bc_sb[:, q0 : q0 + qw],
                                        r_sb[0:1, q0 : q0 + qw],
                                        channels=DH,
                                    )
                                    nc.vector.tensor_mul(
                                        out=ao_sb[
                                            ri * DH : (ri + 1) * DH,
                                            p,
                                            q0 : q0 + qw,
                                        ],
                                        in0=ps_o[0:DH, q0 : q0 + qw],
                                        in1=bc_sb[:, q0 : q0 + qw],
                                    )
                            if enext is not None:
                                ecur[qi] = enext

                for p in range(NPAIR - 1):
                    attn_pair(p)

            # x freed: the wp prefetch DMAs (arena-aliased with x) run
            # during the last pair's attention, and the last pair's slots
            # (which have no Q/K work left) are filled with the first
            # projection groups' partial contractions (kt <= KT-2; the
            # pair's own ao only enters at kt = KT-1).  Projection PSUM
            # comes from psQK -- free once the Q/K stream is exhausted --
            # so no proj matmul ever waits on the attention accumulators.
            with tc.tile_pool(name="wpp", bufs=2) as wpp:
                w_dp = w_d["wp"][:, :].rearrange("p (kt n) -> p kt n", n=D)
                pj_groups = []
                for c0, cw in _chunks(D, 512):
                    wp_sb = wpp.tile([P, KT, 512], BF16, tag="wp")
                    nc.sync.dma_start(
                        out=wp_sb[:, :, 0:cw], in_=w_dp[:, :, c0 : c0 + cw]
                    )
                    for mt in range(MT):
                        pj_groups.append((c0, cw, wp_sb, mt))

                pj = {"gi": 0, "kt": 0, "ps": None}

                def pj_emit(n, during_attn=False):
                    for _ in range(n):
                        gi = pj["gi"]
                        if gi >= len(pj_groups):
                            return
                        if during_attn and (gi >= 2 or pj["kt"] >= KT - 1):
                            return
                        c0, cw, wp_sb, mt = pj_groups[gi]
                        kt = pj["kt"]
                        if kt == 0:
                            ps_p = psQK.tile([P, 512], F32, tag="psQK")
                            pj["ps"] = ps_p
                        nc.tensor.matmul(
                            pj["ps"][:, 0:cw],
                            lhsT=ao_sb[:, kt, mt * P : (mt + 1) * P],
                            rhs=wp_sb[:, kt, 0:cw],
                            start=(kt == 0),
                            stop=(kt == KT - 1),
                        )
                        if kt == KT - 1:
                            o_sb = outp.tile([P, 512], F32, tag="o")
                            nc.vector.tensor_add(
                                out=o_sb[:, 0:cw],
                                in0=pj["ps"][:, 0:cw],
                                in1=bp_bc[:, c0 : c0 + cw],
                            )
                            nc.sync.dma_start(
                                out=out_d[mt * P : (mt + 1) * P, c0 : c0 + cw],
                                in_=o_sb[:, 0:cw],
                            )
                            pj["kt"] = 0
                            pj["gi"] += 1
                        else:
                            pj["kt"] = kt + 1

                attn_pair(NPAIR - 1, filler=lambda n: pj_emit(n, True))
                pj_emit(len(pj_groups) * (KT + 1))

    return nc


# ---------------------------------------------------------------------------
# host-side layout prep
# ---------------------------------------------------------------------------

def _round_f32r(x):
    """RNE to f32r's 11-explicit-mantissa-bit grid (matches HW rounding)."""
    u = np.ascontiguousarray(x, np.float32).view(np.uint32)
    u = ((u + np.uint32(1 << 11)) >> 12) << 12
    return u.view(np.float32)


def host_prep_shared(w_qkv, b_qkv, w_proj, b_proj, D, H):
    """Split/retile the weights once for all cores."""
    KT = D // P
    NPAIR = H // 2

    def tile_w(w):  # [D, N] -> [P, KT*N]
        N = w.shape[1]
        return _round_f32r(
            w.reshape(KT, P, N).transpose(1, 0, 2).reshape(P, KT * N)
        )

    def tile_w_bf16(w):
        import ml_dtypes

        N = w.shape[1]
        return np.ascontiguousarray(
            w.reshape(KT, P, N).transpose(1, 0, 2).reshape(P, KT * N)
        ).astype(ml_dtypes.bfloat16)

    wq3 = w_qkv.reshape(D, H, DH, 3)
    out = {
        "wq": tile_w(np.ascontiguousarray(wq3[:, :, :, 0].reshape(D, D))),
        "wk": tile_w(np.ascontiguousarray(wq3[:, :, :, 1].reshape(D, D))),
        "wv": tile_w_bf16(np.ascontiguousarray(wq3[:, :, :, 2].reshape(D, D))),
        "wp": tile_w_bf16(np.ascontiguousarray(w_proj)),
    }
    b3 = b_qkv.reshape(H, DH, 3)
    bq = np.ascontiguousarray(b3[:, :, 0].reshape(D))
    bk = np.ascontiguousarray(b3[:, :, 1].reshape(D))
    bv = np.ascontiguousarray(b3[:, :, 2].reshape(D))
    out["bq"] = np.ascontiguousarray(bq.reshape(NPAIR, P).T).astype(np.float32)
    out["bk"] = np.ascontiguousarray(bk.reshape(NPAIR, P).T).astype(np.float32)
    import ml_dtypes

    out["bv"] = bv.reshape(1, D).astype(ml_dtypes.bfloat16)
    out["bp"] = np.asarray(b_proj, np.float32).reshape(1, D).astype(
        ml_dtypes.bfloat16
    )
    return out


def host_prep_x(x_b, TOK, D):
    """One batch element [TOK, D] -> x^T tiled [P, KT*TOK]."""
    KT = D // P
    xT = np.ascontiguousarray(np.asarray(x_b, np.float32).T)  # [D, TOK]
    return _round_f32r(
        xT.reshape(KT, P, TOK).transpose(1, 0, 2).reshape(P, KT * TOK)
    )


def host_prep_x_bf16(x_b, TOK, D):
    import ml_dtypes

    KT = D // P
    xT = np.ascontiguousarray(np.asarray(x_b, np.float32).T)
    return (
        xT.reshape(KT, P, TOK)
        .transpose(1, 0, 2)
        .reshape(P, KT * TOK)
        .astype(ml_dtypes.bfloat16)
    )


# ---------------------------------------------------------------------------
# entry point
# ---------------------------------------------------------------------------

_BUILT = {}


def _get_nc(TOK, D, H, att_scale):
    key = (TOK, D, H, att_scale)
    if key not in _BUILT:
        nc = bacc.Bacc(
            "TRN2",
            target_bir_lowering=False,
            debug=False,
            dynamic_dma_scratch_size=512,
        )
        build(nc, TOK, D, H, att_scale)
        nc.compile()
        nc.finalize()
        _BUILT[key] = nc
    return _BUILT[key]


def kernel(x, w_qkv, b_qkv, w_proj, b_proj):
    from concourse.bass_utils import run_bass_kernel_spmd

    x = np.asarray(x, np.float32)
    B, TOK, D = x.shape
    H = H_FULL
    shared = host_prep_shared(
        np.asarray(w_qkv, np.float32),
        np.asarray(b_qkv, np.float32),
        np.asarray(w_proj, np.float32),
        np.asarray(b_proj, np.float32),
        D,
        H,
    )
    in_maps = []
    for b in range(B):
        m = dict(shared)
        m["x"] = host_prep_x(x[b], TOK, D)
        m["xb"] = host_prep_x_bf16(x[b], TOK, D)
        in_maps.append(m)

    nc = _get_nc(TOK, D, H, ATT_SCALE_FULL)
    res = run_bass_kernel_spmd(nc, in_maps, list(range(N_CORES)))
    out = np.stack([res.results[b]["out"] for b in range(B)], axis=0)
    return out.astype(np.float32)



# revision 37
# speedup vs baseline: 1.2179x; 1.1908x over previous
"""Multi-head attention kernel for Trainium2 (Bass/Tile), 8 NeuronCores.

Problem: nn_MultiHeadAttention
  x [8, 1024, 1024] f32, w_qkv [1024, 3072], b_qkv [3072],
  w_proj [1024, 1024], b_proj [1024]  ->  out [8, 1024, 1024]

  qkv = x @ w_qkv + b_qkv ; split (h, d, 3) interleaved on last dim
  score = q k^T per (b, h);  att = softmax(score, -1) / sqrt(1024)
  out = (att @ v) reshaped @ w_proj + b_proj

Sharding: data-parallel over batch. Each of the 8 cores runs the full
MHA for one batch element; no collectives. Host pre-transposes x and
pre-splits w_qkv so the device program is pure matmul + softmax.

Device-side math per core (all layouts chosen so no on-device transpose
is ever needed):
  qT = (x wq)^T  [(h,d), tok]   lhsT=wq tile, rhs=x^T tile
  kT = (x wk)^T  [(h,d), tok]
  v  = x wv      [tok, (h,d)]   + ones-column per head -> v_aug
  per head: S^T[k,q] = kT.T-slice matmul; E = exp(S^T)
            O'^T[0:64,q] ; O'^T[64,q]=sum_k E  via v_aug ones column
            attoutT = O'[0:64] * (scale / O'[64]) (bcast by PE outer-product)
  out = attoutT.T @ wp + bp   (bias via ones outer-product matmul)
"""

import os

os.environ.setdefault("MYCRO_LOCAL_CACHE", "1")

import numpy as np

import concourse.bass as bass
import concourse.tile as tile
from concourse import bacc, mybir

P = 128
DH = 64  # head dim
F32 = mybir.dt.float32
F32R = mybir.dt.float32r
BF16 = mybir.dt.bfloat16
# matmul-operand dtype: float32r streams at full PE rate (4x fp32);
# values are fp32 bit-patterns rounded by the producing engine.
# The attention-probability path (E = exp(S), v) is bf16: probabilities
# are in [0,1] and v feeds only the softmax average, so 8 mantissa bits
# are plenty, and it halves their SBUF footprint.
MM = F32R

# full-problem constants
B_FULL = 8
TOK_FULL = 1024
D_FULL = 1024
H_FULL = 16
ATT_SCALE_FULL = 1.0 / 32.0  # 1/sqrt(1024), applied after softmax
N_CORES = 8


def _chunks(total, step=512):
    return [(s, min(step, total - s)) for s in range(0, total, step)]


def build(nc, TOK, D, H, att_scale):
    """Emit the one-core MHA program (one batch element).

    DRAM inputs (host pre-laid-out):
      x        [P, KT*TOK]   [p, kt, t] = x[t, kt*P + p]   (x^T, kt-tiled)
      wq/wk/wv/wp [P, KT*D]  [p, kt, n] = w[kt*P + p, n]
      bq/bk    [P, NPAIR]    [p, m] = b[m*P + p]
      bv/bp    [1, D]
    Output: out [TOK, D]

    Structure: V phase up front, then per head pair the attention loop
    with the NEXT pair's Q/K matmul groups interleaved into its k-block
    slots, so the PE never idles on the exp (ACT) dependency chain and
    the HAM clock gate stays at full rate.  PSUM budget (8 banks):
    scores 2 x [128,512] = 2, att@v accumulators 2 x [65,TOK] = 4,
    interleaved Q/K group 2 x [128,512] = 2.
    """
    assert D == H * DH and D % P == 0 and TOK % P == 0 and H % 2 == 0
    KT = D // P       # contraction tiles over the model dim
    MT = TOK // P     # token tiles
    NPAIR = H // 2    # head pairs (== D // P)
    VW = H * (DH + 1)  # v_aug row width: per head [v | 1]
    EXP = mybir.ActivationFunctionType.Exp

    x_d = nc.dram_tensor("x", [P, KT * TOK], MM, kind="ExternalInput")
    xb_d = nc.dram_tensor("xb", [P, KT * TOK], BF16, kind="ExternalInput")
    w_d = {}
    for nm in ("wq", "wk"):
        w_d[nm] = nc.dram_tensor(nm, [P, KT * D], MM, kind="ExternalInput")
    for nm in ("wv", "wp"):
        w_d[nm] = nc.dram_tensor(nm, [P, KT * D], BF16, kind="ExternalInput")
    bq_d = nc.dram_tensor("bq", [P, NPAIR], F32, kind="ExternalInput")
    bk_d = nc.dram_tensor("bk", [P, NPAIR], F32, kind="ExternalInput")
    bv_d = nc.dram_tensor("bv", [1, D], BF16, kind="ExternalInput")
    bp_d = nc.dram_tensor("bp", [1, D], BF16, kind="ExternalInput")
    out_d = nc.dram_tensor("out", [TOK, D], F32, kind="ExternalOutput")

    QH = 512 if TOK >= 512 else TOK  # q-stripe width (PSUM bank = 512 f32)

    with tile.TileContext(nc) as tc:
        with (
            tc.tile_pool(name="sing", bufs=1) as sing,
            tc.tile_pool(name="psS", bufs=2, space="PSUM") as psS,
            tc.tile_pool(name="psQK", bufs=2, space="PSUM") as psQK,
            tc.tile_pool(name="psB", bufs=2, space="PSUM") as psB,
            tc.tile_pool(name="ebuf", bufs=8) as ebuf,
            tc.tile_pool(name="rbuf", bufs=2) as rbuf,
            tc.tile_pool(name="bcp", bufs=2) as bcp,
            tc.tile_pool(name="outp", bufs=2) as outp,
            tc.tile_pool(name="wqkp", bufs=3) as wqkp,
        ):
            # ---------------- persistent SBUF ----------------
            # v_aug denominator columns carry 1/att_scale so the softmax
            # denominator comes out pre-divided by att_scale: recip of it
            # directly yields att_scale / sum(exp)
            vones_sb = sing.tile([P, MT * H], F32, tag="vones")
            nc.vector.memset(vones_sb, 1.0 / att_scale)

            bq_sb = sing.tile([P, NPAIR], F32, tag="bq")
            nc.sync.dma_start(out=bq_sb, in_=bq_d[:, :])
            bk_sb = sing.tile([P, NPAIR], F32, tag="bk")
            nc.sync.dma_start(out=bk_sb, in_=bk_d[:, :])
            # biases enter via DVE adds fused into the existing PSUM
            # drains (no PE outer-product matmuls): broadcast them across
            # partitions once on the idle GPSIMD engine
            bv_sb = sing.tile([1, D], BF16, tag="bv")
            nc.sync.dma_start(out=bv_sb, in_=bv_d[:, :])
            bp_sb = sing.tile([1, D], BF16, tag="bp")
            nc.sync.dma_start(out=bp_sb, in_=bp_d[:, :])
            bv_bc = sing.tile([P, D], BF16, tag="bvbc")
            nc.gpsimd.partition_broadcast(bv_bc[:, :], bv_sb[0:1, :], channels=P)
            bp_bc = sing.tile([P, D], BF16, tag="bpbc")
            nc.gpsimd.partition_broadcast(bp_bc[:, :], bp_sb[0:1, :], channels=P)

            v_sb = sing.tile([P, MT, VW], BF16, tag="v")     # v_aug
            nc.vector.tensor_copy(
                out=v_sb[:, :, :]
                .rearrange("p m (h e) -> p m h e", e=DH + 1)[:, :, :, DH],
                in_=vones_sb[:, :].rearrange("p (m h) -> p m h", h=H),
            )
            qT_sb = sing.tile([P, NPAIR, TOK], MM, tag="qT")
            kT_sb = sing.tile([P, NPAIR, TOK], MM, tag="kT")
            ao_sb = sing.tile([P, NPAIR, TOK], BF16, tag="ao")  # attout^T

            with tc.tile_pool(name="xp", bufs=1) as xp:
                x_sb = xp.tile([P, KT * TOK], MM, tag="x")
                x3 = x_sb[:, :].rearrange("p (kt t) -> p kt t", t=TOK)
                x_d3 = x_d[:, :].rearrange("p (kt t) -> p kt t", t=TOK)
                w_d3 = {
                    nm: w_d[nm][:, :].rearrange("p (kt n) -> p kt n", n=D)
                    for nm in w_d
                }

                # per-pair Q/K weight tiles, DMA'd one pair ahead
                w_tiles = {}

                def fetch_qk_pair(pg):
                    for wname in ("wq", "wk"):
                        wt = wqkp.tile([P, KT, P], MM, tag="w" + str(pg % 2))
                        nc.sync.dma_start(
                            out=wt,
                            in_=w_d3[wname][:, :, pg * P : (pg + 1) * P],
                        )
                        w_tiles[(wname, pg)] = wt

                # Q/K matmul-group stream, interleaved into attention slots
                qk_state = {"groups": [], "gi": 0, "kt": 0, "ps": None}

                def qk_add_pair(pg):
                    for wname, dst_sb, b_sb in (
                        ("wq", qT_sb, bq_sb),
                        ("wk", kT_sb, bk_sb),
                    ):
                        for c0, cw in _chunks(TOK, QH):
                            qk_state["groups"].append(
                                (wname, pg, c0, cw, dst_sb, b_sb)
                            )

                def qk_emit(n):
                    for _ in range(n):
                        if qk_state["gi"] >= len(qk_state["groups"]):
                            return
                        wname, pg, c0, cw, dst_sb, b_sb = qk_state["groups"][
                            qk_state["gi"]
                        ]
                        kt = qk_state["kt"]
                        if kt == 0:
                            ps_qk = psQK.tile([P, QH], F32, tag="psQK")
                            qk_state["ps"] = ps_qk
                        nc.tensor.matmul(
                            qk_state["ps"][:, 0:cw],
                            lhsT=w_tiles[(wname, pg)][:, kt, :],
                            rhs=x3[:, kt, c0 : c0 + cw],
                            start=(kt == 0),
                            stop=(kt == KT - 1),
                        )
                        if kt == KT - 1:
                            nc.vector.tensor_scalar_add(
                                out=dst_sb[:, pg, c0 : c0 + cw],
                                in0=qk_state["ps"][:, 0:cw],
                                scalar1=b_sb[:, pg : pg + 1],
                            )
                            qk_state["kt"] = 0
                            qk_state["gi"] += 1
                        else:
                            qk_state["kt"] = kt + 1

                # ---------------- DMA prologue ----------------
                # The V phase runs on bf16 copies of x and wv (half the
                # bytes on the DMA-bound critical path).  Dependency
                # tracking is tile-granular, so each token-quarter of xb
                # and column-chunk of wv gets its OWN tile: the first V
                # matmul then waits only on the two small DMAs it reads.
                with tc.tile_pool(name="wvp", bufs=1) as wvp:
                    xb_d3 = xb_d[:, :].rearrange("p (kt t) -> p kt t", t=TOK)
                    vchunks = _chunks(D, 512)
                    xq_w = max(P, TOK // 4)
                    mpq = xq_w // P  # mt-blocks per xb quarter tile
                    wv_tiles = []
                    xb_tiles = []
                    for ci, (c0, cw) in enumerate(vchunks):
                        wvt = wvp.tile([P, KT, cw], BF16, tag="wv" + str(ci))
                        wv_tiles.append(wvt)
                    for qq in range(TOK // xq_w):
                        xbt = wvp.tile([P, KT, xq_w], BF16, tag="xq" + str(qq))
                        xb_tiles.append(xbt)
                    nc.sync.dma_start(
                        out=wv_tiles[0],
                        in_=w_d3["wv"][:, :, 0 : vchunks[0][1]],
                    )
                    nc.sync.dma_start(
                        out=xb_tiles[0], in_=xb_d3[:, :, 0:xq_w]
                    )
                    for ci, (c0, cw) in list(enumerate(vchunks))[1:]:
                        nc.sync.dma_start(
                            out=wv_tiles[ci], in_=w_d3["wv"][:, :, c0 : c0 + cw]
                        )
                    for qq in range(1, TOK // xq_w):
                        nc.sync.dma_start(
                            out=xb_tiles[qq],
                            in_=xb_d3[:, :, qq * xq_w : (qq + 1) * xq_w],
                        )
                    # full-precision x (for Q/K) streams in behind the bf16
                    # copies; it is only needed once pair-0 Q/K starts
                    nc.sync.dma_start(
                        out=x_sb[:, :], in_=x_d[:, :]
                    )
                    fetch_qk_pair(0)

                    # ------------ V phase: v = x wv + bv (natural) ----
                    # mt-major so compute follows the token-quarter DMAs
                    for mt in range(MT):
                        for ci, (c0, cw) in enumerate(vchunks):
                            ps_v = psS.tile([P, QH], F32, tag="psS")
                            for kt in range(KT):
                                nc.tensor.matmul(
                                    ps_v[:, 0:cw],
                                    lhsT=xb_tiles[mt // mpq][
                                        :, kt, (mt % mpq) * P : (mt % mpq + 1) * P
                                    ],
                                    rhs=wv_tiles[ci][:, kt, 0:cw],
                                    start=(kt == 0),
                                    stop=(kt == KT - 1),
                                )
                            # scatter heads into v_aug (65-stride), adding
                            # bv during the drain
                            nh = cw // DH
                            h0 = c0 // DH
                            nc.vector.tensor_add(
                                out=v_sb[:, mt, :]
                                .rearrange("p (h e) -> p h e", e=DH + 1)[
                                    :, h0 : h0 + nh, 0:DH
                                ],
                                in0=ps_v[:, 0:cw].rearrange(
                                    "p (h d) -> p h d", d=DH
                                ),
                                in1=bv_bc[:, c0 : c0 + cw].rearrange(
                                    "p (h d) -> p h d", d=DH
                                ),
                            )

                # ---------------- pair-0 Q/K, up front ----------------
                qk_add_pair(0)
                qk_emit(4 * KT)

                # -------- attention + interleaved next-pair Q/K -------
                def emit_scores_half(p, kb, q0, qw):
                    """Both heads' scores for one q stripe; the two matmuls
                    land in different PE row groups (base partitions 0/64)
                    and run concurrently."""
                    pss = []
                    for base in (0, DH):
                        ps = psS.tile([P, QH], F32, tag="psS")
                        nc.tensor.matmul(
                            ps[:, 0:qw],
                            lhsT=kT_sb[
                                base : base + DH, p, kb * P : (kb + 1) * P
                            ],
                            rhs=qT_sb[base : base + DH, p, q0 : q0 + qw],
                            start=True,
                            stop=True,
                        )
                        pss.append(ps)
                    ets = []
                    for ps in pss:
                        et = ebuf.tile([P, QH], BF16, tag="E")
                        nc.scalar.activation(
                            out=et[:, 0:qw], in_=ps[:, 0:qw], func=EXP
                        )
                        ets.append(et)
                    return ets

                def attn_pair(p, filler=None):
                    if p + 1 < NPAIR:
                        fetch_qk_pair(p + 1)
                        qk_add_pair(p + 1)
                    ps_oe = psB.tile([DH + 1, TOK], F32, tag="psB")
                    ps_oo = psB.tile([DH + 1, TOK], F32, tag="psB")
                    nrm = {}

                    qhalves = _chunks(TOK, QH)
                    ecur = [emit_scores_half(p, 0, q0, qw) for q0, qw in qhalves]
                    for kb in range(MT):
                        for qi, (q0, qw) in enumerate(qhalves):
                            enext = None
                            if kb + 1 < MT:
                                enext = emit_scores_half(p, kb + 1, q0, qw)
                            # Q/K filler sits between the scores and the
                            # att@v consumers so exp(kb) has ~1.3us more
                            # lead time before att@v needs its output
                            qk_emit(2)
                            if filler is not None:
                                filler(2)
                            for hoff, ps_out in ((0, ps_oe), (1, ps_oo)):
                                hh = 2 * p + hoff
                                nc.tensor.matmul(
                                    ps_out[:, q0 : q0 + qw],
                                    lhsT=v_sb[
                                        :,
                                        kb,
                                        hh * (DH + 1) : (hh + 1) * (DH + 1),
                                    ],
                                    rhs=ecur[qi][hoff][:, 0:qw],
                                    start=(kb == 0),
                                    stop=(kb == MT - 1),
                                    skip_group_check=True,
                                )
                            if kb == MT - 1:
                                # normalize this q stripe as soon as its
                                # accumulation closes:
                                #   attout^T = O'[0:DH] * recip(O'[DH])
                                # (att_scale is folded into the v_aug ones
                                # column).  reciprocal_approx_fast silently
                                # corrupts on HW when its input AP starts at
                                # partition 64, so stage the denominator row
                                # at partition 0 first (row 0 of the bc
                                # tile, which the broadcast then overwrites
                                # -- its RAW dep on r guarantees the recip
                                # consumed it first).
                                if qi == 0:
                                    r_e = rbuf.tile([1, TOK], F32, tag="R")
                                    r_o = rbuf.tile([1, TOK], F32, tag="R")
                                    bc_e = bcp.tile([DH, TOK], F32, tag="BC")
                                    bc_o = bcp.tile([DH, TOK], F32, tag="BC")
                                    nrm[0] = (ps_oe, r_e, bc_e)
                                    nrm[1] = (ps_oo, r_o, bc_o)
                                for ri in (0, 1):
                                    ps_o, r_sb, bc_sb = nrm[ri]
                                    nc.vector.tensor_copy(
                                        out=bc_sb[0:1, q0 : q0 + qw],
                                        in_=ps_o[DH : DH + 1, q0 : q0 + qw],
                                    )
                                    nc.vector.reciprocal_approx_fast(
                                        out=r_sb[0:1, q0 : q0 + qw],
                                        in_=bc_sb[0:1, q0 : q0 + qw],
                                    )
                                    nc.gpsimd.partition_broadcast(
                                        bc_sb[:, q0 : q0 + qw],
                                        r_sb[0:1, q0 : q0 + qw],
                                        channels=DH,
                                    )
                                    nc.vector.tensor_mul(
                                        out=ao_sb[
                                            ri * DH : (ri + 1) * DH,
                                            p,
                                            q0 : q0 + qw,
                                        ],
                                        in0=ps_o[0:DH, q0 : q0 + qw],
                                        in1=bc_sb[:, q0 : q0 + qw],
                                    )
                            if enext is not None:
                                ecur[qi] = enext

                for p in range(NPAIR - 1):
                    attn_pair(p)

            # x freed: the wp prefetch DMAs (arena-aliased with x) run
            # during the last pair's attention, and the last pair's slots
            # (which have no Q/K work left) are filled with the first
            # projection groups' partial contractions (kt <= KT-2; the
            # pair's own ao only enters at kt = KT-1).  Projection PSUM
            # comes from psQK -- free once the Q/K stream is exhausted --
            # so no proj matmul ever waits on the attention accumulators.
            with tc.tile_pool(name="wpp", bufs=2) as wpp:
                w_dp = w_d["wp"][:, :].rearrange("p (kt n) -> p kt n", n=D)
                pj_groups = []
                for c0, cw in _chunks(D, 512):
                    wp_sb = wpp.tile([P, KT, 512], BF16, tag="wp")
                    nc.sync.dma_start(
                        out=wp_sb[:, :, 0:cw], in_=w_dp[:, :, c0 : c0 + cw]
                    )
                    for mt in range(MT):
                        pj_groups.append((c0, cw, wp_sb, mt))

                pj = {"gi": 0, "kt": 0, "ps": None}

                def pj_emit(n, during_attn=False):
                    for _ in range(n):
                        gi = pj["gi"]
                        if gi >= len(pj_groups):
                            return
                        if during_attn and (gi >= 2 or pj["kt"] >= KT - 1):
                            return
                        c0, cw, wp_sb, mt = pj_groups[gi]
                        kt = pj["kt"]
                        if kt == 0:
                            ps_p = psQK.tile([P, 512], F32, tag="psQK")
                            pj["ps"] = ps_p
                        nc.tensor.matmul(
                            pj["ps"][:, 0:cw],
                            lhsT=ao_sb[:, kt, mt * P : (mt + 1) * P],
                            rhs=wp_sb[:, kt, 0:cw],
                            start=(kt == 0),
                            stop=(kt == KT - 1),
                        )
                        if kt == KT - 1:
                            o_sb = outp.tile([P, 512], F32, tag="o")
                            nc.vector.tensor_add(
                                out=o_sb[:, 0:cw],
                                in0=pj["ps"][:, 0:cw],
                                in1=bp_bc[:, c0 : c0 + cw],
                            )
                            nc.sync.dma_start(
                                out=out_d[mt * P : (mt + 1) * P, c0 : c0 + cw],
                                in_=o_sb[:, 0:cw],
                            )
                            pj["kt"] = 0
                            pj["gi"] += 1
                        else:
                            pj["kt"] = kt + 1

                attn_pair(NPAIR - 1, filler=lambda n: pj_emit(n, True))
                pj_emit(len(pj_groups) * (KT + 1))

    return nc


# ---------------------------------------------------------------------------
# host-side layout prep
# ---------------------------------------------------------------------------

def _round_f32r(x):
    """RNE to f32r's 11-explicit-mantissa-bit grid (matches HW rounding)."""
    u = np.ascontiguousarray(x, np.float32).view(np.uint32)
    u = ((u + np.uint32(1 << 11)) >> 12) << 12
    return u.view(np.float32)


def host_prep_shared(w_qkv, b_qkv, w_proj, b_proj, D, H):
    """Split/retile the weights once for all cores."""
    KT = D // P
    NPAIR = H // 2

    def tile_w(w):  # [D, N] -> [P, KT*N]
        N = w.shape[1]
        return _round_f32r(
            w.reshape(KT, P, N).transpose(1, 0, 2).reshape(P, KT * N)
        )

    def tile_w_bf16(w):
        import ml_dtypes

        N = w.shape[1]
        return np.ascontiguousarray(
            w.reshape(KT, P, N).transpose(1, 0, 2).reshape(P, KT * N)
        ).astype(ml_dtypes.bfloat16)

    wq3 = w_qkv.reshape(D, H, DH, 3)
    out = {
        "wq": tile_w(np.ascontiguousarray(wq3[:, :, :, 0].reshape(D, D))),
        "wk": tile_w(np.ascontiguousarray(wq3[:, :, :, 1].reshape(D, D))),
        "wv": tile_w_bf16(np.ascontiguousarray(wq3[:, :, :, 2].reshape(D, D))),
        "wp": tile_w_bf16(np.ascontiguousarray(w_proj)),
    }
    b3 = b_qkv.reshape(H, DH, 3)
    bq = np.ascontiguousarray(b3[:, :, 0].reshape(D))
    bk = np.ascontiguousarray(b3[:, :, 1].reshape(D))
    bv = np.ascontiguousarray(b3[:, :, 2].reshape(D))
    out["bq"] = np.ascontiguousarray(bq.reshape(NPAIR, P).T).astype(np.float32)
    out["bk"] = np.ascontiguousarray(bk.reshape(NPAIR, P).T).astype(np.float32)
    import ml_dtypes

    out["bv"] = bv.reshape(1, D).astype(ml_dtypes.bfloat16)
    out["bp"] = np.asarray(b_proj, np.float32).reshape(1, D).astype(
        ml_dtypes.bfloat16
    )
    return out


def host_prep_x(x_b, TOK, D):
    """One batch element [TOK, D] -> x^T tiled [P, KT*TOK]."""
    KT = D // P
    xT = np.ascontiguousarray(np.asarray(x_b, np.float32).T)  # [D, TOK]
    return _round_f32r(
        xT.reshape(KT, P, TOK).transpose(1, 0, 2).reshape(P, KT * TOK)
    )


def host_prep_x_bf16(x_b, TOK, D):
    import ml_dtypes

    KT = D // P
    xT = np.ascontiguousarray(np.asarray(x_b, np.float32).T)
    return (
        xT.reshape(KT, P, TOK)
        .transpose(1, 0, 2)
        .reshape(P, KT * TOK)
        .astype(ml_dtypes.bfloat16)
    )


# ---------------------------------------------------------------------------
# entry point
# ---------------------------------------------------------------------------

_BUILT = {}


def _get_nc(TOK, D, H, att_scale):
    key = (TOK, D, H, att_scale)
    if key not in _BUILT:
        nc = bacc.Bacc(
            "TRN2",
            target_bir_lowering=False,
            debug=False,
            dynamic_dma_scratch_size=512,
        )
        build(nc, TOK, D, H, att_scale)
        nc.compile()
        nc.finalize()
        _BUILT[key] = nc
    return _BUILT[key]


def kernel(x, w_qkv, b_qkv, w_proj, b_proj):
    from concourse.bass_utils import run_bass_kernel_spmd

    x = np.asarray(x, np.float32)
    B, TOK, D = x.shape
    H = H_FULL
    shared = host_prep_shared(
        np.asarray(w_qkv, np.float32),
        np.asarray(b_qkv, np.float32),
        np.asarray(w_proj, np.float32),
        np.asarray(b_proj, np.float32),
        D,
        H,
    )
    in_maps = []
    for b in range(B):
        m = dict(shared)
        m["x"] = host_prep_x(x[b], TOK, D)
        m["xb"] = host_prep_x_bf16(x[b], TOK, D)
        in_maps.append(m)

    nc = _get_nc(TOK, D, H, ATT_SCALE_FULL)
    res = run_bass_kernel_spmd(nc, in_maps, list(range(N_CORES)))
    out = np.stack([res.results[b]["out"] for b in range(B)], axis=0)
    return out.astype(np.float32)



# revision 38
# speedup vs baseline: 1.2219x; 1.0032x over previous
"""Multi-head attention kernel for Trainium2 (Bass/Tile), 8 NeuronCores.

Problem: nn_MultiHeadAttention
  x [8, 1024, 1024] f32, w_qkv [1024, 3072], b_qkv [3072],
  w_proj [1024, 1024], b_proj [1024]  ->  out [8, 1024, 1024]

  qkv = x @ w_qkv + b_qkv ; split (h, d, 3) interleaved on last dim
  score = q k^T per (b, h);  att = softmax(score, -1) / sqrt(1024)
  out = (att @ v) reshaped @ w_proj + b_proj

Sharding: data-parallel over batch. Each of the 8 cores runs the full
MHA for one batch element; no collectives. Host pre-transposes x and
pre-splits w_qkv so the device program is pure matmul + softmax.

Device-side math per core (all layouts chosen so no on-device transpose
is ever needed):
  qT = (x wq)^T  [(h,d), tok]   lhsT=wq tile, rhs=x^T tile
  kT = (x wk)^T  [(h,d), tok]
  v  = x wv      [tok, (h,d)]   + ones-column per head -> v_aug
  per head: S^T[k,q] = kT.T-slice matmul; E = exp(S^T)
            O'^T[0:64,q] ; O'^T[64,q]=sum_k E  via v_aug ones column
            attoutT = O'[0:64] * (scale / O'[64]) (bcast by PE outer-product)
  out = attoutT.T @ wp + bp   (bias via ones outer-product matmul)
"""

import os

os.environ.setdefault("MYCRO_LOCAL_CACHE", "1")

import numpy as np

import concourse.bass as bass
import concourse.tile as tile
from concourse import bacc, mybir

P = 128
DH = 64  # head dim
F32 = mybir.dt.float32
F32R = mybir.dt.float32r
BF16 = mybir.dt.bfloat16
# matmul-operand dtype: float32r streams at full PE rate (4x fp32);
# values are fp32 bit-patterns rounded by the producing engine.
# The attention-probability path (E = exp(S), v) is bf16: probabilities
# are in [0,1] and v feeds only the softmax average, so 8 mantissa bits
# are plenty, and it halves their SBUF footprint.
MM = F32R

# full-problem constants
B_FULL = 8
TOK_FULL = 1024
D_FULL = 1024
H_FULL = 16
ATT_SCALE_FULL = 1.0 / 32.0  # 1/sqrt(1024), applied after softmax
N_CORES = 8


def _chunks(total, step=512):
    return [(s, min(step, total - s)) for s in range(0, total, step)]


def build(nc, TOK, D, H, att_scale):
    """Emit the one-core MHA program (one batch element).

    DRAM inputs (host pre-laid-out):
      x        [P, KT*TOK]   [p, kt, t] = x[t, kt*P + p]   (x^T, kt-tiled)
      wq/wk/wv/wp [P, KT*D]  [p, kt, n] = w[kt*P + p, n]
      bq/bk    [P, NPAIR]    [p, m] = b[m*P + p]
      bv/bp    [1, D]
    Output: out [TOK, D]

    Structure: V phase up front, then per head pair the attention loop
    with the NEXT pair's Q/K matmul groups interleaved into its k-block
    slots, so the PE never idles on the exp (ACT) dependency chain and
    the HAM clock gate stays at full rate.  PSUM budget (8 banks):
    scores 2 x [128,512] = 2, att@v accumulators 2 x [65,TOK] = 4,
    interleaved Q/K group 2 x [128,512] = 2.
    """
    assert D == H * DH and D % P == 0 and TOK % P == 0 and H % 2 == 0
    KT = D // P       # contraction tiles over the model dim
    MT = TOK // P     # token tiles
    NPAIR = H // 2    # head pairs (== D // P)
    VW = H * (DH + 1)  # v_aug row width: per head [v | 1]
    EXP = mybir.ActivationFunctionType.Exp

    x_d = nc.dram_tensor("x", [P, KT * TOK], MM, kind="ExternalInput")
    xb_d = nc.dram_tensor("xb", [P, KT * TOK], BF16, kind="ExternalInput")
    w_d = {}
    for nm in ("wq", "wk"):
        w_d[nm] = nc.dram_tensor(nm, [P, KT * D], MM, kind="ExternalInput")
    for nm in ("wv", "wp"):
        w_d[nm] = nc.dram_tensor(nm, [P, KT * D], BF16, kind="ExternalInput")
    bq_d = nc.dram_tensor("bq", [P, NPAIR], F32, kind="ExternalInput")
    bk_d = nc.dram_tensor("bk", [P, NPAIR], F32, kind="ExternalInput")
    bv_d = nc.dram_tensor("bv", [1, D], BF16, kind="ExternalInput")
    bp_d = nc.dram_tensor("bp", [1, D], BF16, kind="ExternalInput")
    out_d = nc.dram_tensor("out", [TOK, D], F32, kind="ExternalOutput")

    QH = 512 if TOK >= 512 else TOK  # q-stripe width (PSUM bank = 512 f32)

    with tile.TileContext(nc) as tc:
        with (
            tc.tile_pool(name="sing", bufs=1) as sing,
            tc.tile_pool(name="psS", bufs=2, space="PSUM") as psS,
            tc.tile_pool(name="psQK", bufs=2, space="PSUM") as psQK,
            tc.tile_pool(name="psB", bufs=2, space="PSUM") as psB,
            tc.tile_pool(name="ebuf", bufs=8) as ebuf,
            tc.tile_pool(name="rbuf", bufs=2) as rbuf,
            tc.tile_pool(name="bcp", bufs=2) as bcp,
            tc.tile_pool(name="outp", bufs=2) as outp,
            tc.tile_pool(name="wqkp", bufs=3) as wqkp,
        ):
            # ---------------- persistent SBUF ----------------
            # v_aug denominator columns carry 1/att_scale so the softmax
            # denominator comes out pre-divided by att_scale: recip of it
            # directly yields att_scale / sum(exp)
            vones_sb = sing.tile([P, MT * H], F32, tag="vones")
            nc.vector.memset(vones_sb, 1.0 / att_scale)

            bq_sb = sing.tile([P, NPAIR], F32, tag="bq")
            nc.sync.dma_start(out=bq_sb, in_=bq_d[:, :])
            bk_sb = sing.tile([P, NPAIR], F32, tag="bk")
            nc.sync.dma_start(out=bk_sb, in_=bk_d[:, :])
            # biases enter via DVE adds fused into the existing PSUM
            # drains (no PE outer-product matmuls): broadcast them across
            # partitions once on the idle GPSIMD engine
            bv_sb = sing.tile([1, D], BF16, tag="bv")
            nc.sync.dma_start(out=bv_sb, in_=bv_d[:, :])
            bp_sb = sing.tile([1, D], BF16, tag="bp")
            nc.sync.dma_start(out=bp_sb, in_=bp_d[:, :])
            bv_bc = sing.tile([P, D], BF16, tag="bvbc")
            nc.gpsimd.partition_broadcast(bv_bc[:, :], bv_sb[0:1, :], channels=P)
            bp_bc = sing.tile([P, D], BF16, tag="bpbc")
            nc.gpsimd.partition_broadcast(bp_bc[:, :], bp_sb[0:1, :], channels=P)

            v_sb = sing.tile([P, MT, VW], BF16, tag="v")     # v_aug
            nc.vector.tensor_copy(
                out=v_sb[:, :, :]
                .rearrange("p m (h e) -> p m h e", e=DH + 1)[:, :, :, DH],
                in_=vones_sb[:, :].rearrange("p (m h) -> p m h", h=H),
            )
            qT_sb = sing.tile([P, NPAIR, TOK], MM, tag="qT")
            kT_sb = sing.tile([P, NPAIR, TOK], MM, tag="kT")
            ao_sb = sing.tile([P, NPAIR, TOK], BF16, tag="ao")  # attout^T

            with tc.tile_pool(name="xp", bufs=1) as xp:
                x_sb = xp.tile([P, KT * TOK], MM, tag="x")
                x3 = x_sb[:, :].rearrange("p (kt t) -> p kt t", t=TOK)
                x_d3 = x_d[:, :].rearrange("p (kt t) -> p kt t", t=TOK)
                w_d3 = {
                    nm: w_d[nm][:, :].rearrange("p (kt n) -> p kt n", n=D)
                    for nm in w_d
                }

                # per-pair Q/K weight tiles, DMA'd one pair ahead
                w_tiles = {}

                def fetch_qk_pair(pg):
                    for wname in ("wq", "wk"):
                        wt = wqkp.tile([P, KT, P], MM, tag="w" + str(pg % 2))
                        nc.sync.dma_start(
                            out=wt,
                            in_=w_d3[wname][:, :, pg * P : (pg + 1) * P],
                        )
                        w_tiles[(wname, pg)] = wt

                # Q/K matmul-group stream, interleaved into attention slots
                qk_state = {"groups": [], "gi": 0, "kt": 0, "ps": None}

                def qk_add_pair(pg):
                    for wname, dst_sb, b_sb in (
                        ("wq", qT_sb, bq_sb),
                        ("wk", kT_sb, bk_sb),
                    ):
                        for c0, cw in _chunks(TOK, QH):
                            qk_state["groups"].append(
                                (wname, pg, c0, cw, dst_sb, b_sb)
                            )

                def qk_emit(n):
                    for _ in range(n):
                        if qk_state["gi"] >= len(qk_state["groups"]):
                            return
                        wname, pg, c0, cw, dst_sb, b_sb = qk_state["groups"][
                            qk_state["gi"]
                        ]
                        kt = qk_state["kt"]
                        if kt == 0:
                            ps_qk = psQK.tile([P, QH], F32, tag="psQK")
                            qk_state["ps"] = ps_qk
                        nc.tensor.matmul(
                            qk_state["ps"][:, 0:cw],
                            lhsT=w_tiles[(wname, pg)][:, kt, :],
                            rhs=x3[:, kt, c0 : c0 + cw],
                            start=(kt == 0),
                            stop=(kt == KT - 1),
                        )
                        if kt == KT - 1:
                            nc.vector.tensor_scalar_add(
                                out=dst_sb[:, pg, c0 : c0 + cw],
                                in0=qk_state["ps"][:, 0:cw],
                                scalar1=b_sb[:, pg : pg + 1],
                            )
                            qk_state["kt"] = 0
                            qk_state["gi"] += 1
                        else:
                            qk_state["kt"] = kt + 1

                # ---------------- DMA prologue ----------------
                # The V phase runs on bf16 copies of x and wv (half the
                # bytes on the DMA-bound critical path).  Dependency
                # tracking is tile-granular, so each token-quarter of xb
                # and column-chunk of wv gets its OWN tile: the first V
                # matmul then waits only on the two small DMAs it reads.
                with tc.tile_pool(name="wvp", bufs=1) as wvp:
                    xb_d3 = xb_d[:, :].rearrange("p (kt t) -> p kt t", t=TOK)
                    vchunks = _chunks(D, 512)
                    xq_w = max(P, TOK // 4)
                    mpq = xq_w // P  # mt-blocks per xb quarter tile
                    wv_tiles = []
                    xb_tiles = []
                    for ci, (c0, cw) in enumerate(vchunks):
                        wvt = wvp.tile([P, KT, cw], BF16, tag="wv" + str(ci))
                        wv_tiles.append(wvt)
                    for qq in range(TOK // xq_w):
                        xbt = wvp.tile([P, KT, xq_w], BF16, tag="xq" + str(qq))
                        xb_tiles.append(xbt)
                    nc.sync.dma_start(
                        out=wv_tiles[0],
                        in_=w_d3["wv"][:, :, 0 : vchunks[0][1]],
                    )
                    nc.sync.dma_start(
                        out=xb_tiles[0], in_=xb_d3[:, :, 0:xq_w]
                    )
                    for ci, (c0, cw) in list(enumerate(vchunks))[1:]:
                        nc.sync.dma_start(
                            out=wv_tiles[ci], in_=w_d3["wv"][:, :, c0 : c0 + cw]
                        )
                    for qq in range(1, TOK // xq_w):
                        nc.sync.dma_start(
                            out=xb_tiles[qq],
                            in_=xb_d3[:, :, qq * xq_w : (qq + 1) * xq_w],
                        )
                    # full-precision x (for Q/K) streams in behind the bf16
                    # copies; it is only needed once pair-0 Q/K starts
                    nc.sync.dma_start(
                        out=x_sb[:, :], in_=x_d[:, :]
                    )
                    fetch_qk_pair(0)

                    # ------------ V phase: v = x wv + bv (natural) ----
                    # mt-major so compute follows the token-quarter DMAs
                    for mt in range(MT):
                        for ci, (c0, cw) in enumerate(vchunks):
                            ps_v = psS.tile([P, QH], F32, tag="psS")
                            for kt in range(KT):
                                nc.tensor.matmul(
                                    ps_v[:, 0:cw],
                                    lhsT=xb_tiles[mt // mpq][
                                        :, kt, (mt % mpq) * P : (mt % mpq + 1) * P
                                    ],
                                    rhs=wv_tiles[ci][:, kt, 0:cw],
                                    start=(kt == 0),
                                    stop=(kt == KT - 1),
                                )
                            # scatter heads into v_aug (65-stride), adding
                            # bv during the drain
                            nh = cw // DH
                            h0 = c0 // DH
                            nc.vector.tensor_add(
                                out=v_sb[:, mt, :]
                                .rearrange("p (h e) -> p h e", e=DH + 1)[
                                    :, h0 : h0 + nh, 0:DH
                                ],
                                in0=ps_v[:, 0:cw].rearrange(
                                    "p (h d) -> p h d", d=DH
                                ),
                                in1=bv_bc[:, c0 : c0 + cw].rearrange(
                                    "p (h d) -> p h d", d=DH
                                ),
                            )

                # ---------------- pair-0 Q/K, up front ----------------
                qk_add_pair(0)
                qk_emit(4 * KT)

                # -------- attention + interleaved next-pair Q/K -------
                def emit_scores_half(p, kb, q0, qw):
                    """Both heads' scores for one q stripe; the two matmuls
                    land in different PE row groups (base partitions 0/64)
                    and run concurrently."""
                    pss = []
                    for base in (0, DH):
                        ps = psS.tile([P, QH], F32, tag="psS")
                        nc.tensor.matmul(
                            ps[:, 0:qw],
                            lhsT=kT_sb[
                                base : base + DH, p, kb * P : (kb + 1) * P
                            ],
                            rhs=qT_sb[base : base + DH, p, q0 : q0 + qw],
                            start=True,
                            stop=True,
                        )
                        pss.append(ps)
                    ets = []
                    for ps in pss:
                        et = ebuf.tile([P, QH], BF16, tag="E")
                        nc.scalar.activation(
                            out=et[:, 0:qw], in_=ps[:, 0:qw], func=EXP
                        )
                        ets.append(et)
                    return ets

                def attn_pair(p, filler=None):
                    if p + 1 < NPAIR:
                        fetch_qk_pair(p + 1)
                        qk_add_pair(p + 1)
                    ps_oe = psB.tile([DH + 1, TOK], F32, tag="psB")
                    ps_oo = psB.tile([DH + 1, TOK], F32, tag="psB")
                    nrm = {}

                    qhalves = _chunks(TOK, QH)
                    ecur = [emit_scores_half(p, 0, q0, qw) for q0, qw in qhalves]
                    for kb in range(MT):
                        for qi, (q0, qw) in enumerate(qhalves):
                            enext = None
                            if kb + 1 < MT:
                                enext = emit_scores_half(p, kb + 1, q0, qw)
                            for hoff, ps_out in ((0, ps_oe), (1, ps_oo)):
                                hh = 2 * p + hoff
                                nc.tensor.matmul(
                                    ps_out[:, q0 : q0 + qw],
                                    lhsT=v_sb[
                                        :,
                                        kb,
                                        hh * (DH + 1) : (hh + 1) * (DH + 1),
                                    ],
                                    rhs=ecur[qi][hoff][:, 0:qw],
                                    start=(kb == 0),
                                    stop=(kb == MT - 1),
                                    skip_group_check=True,
                                )
                            qk_emit(2)
                            if filler is not None:
                                filler(2)
                            if kb == MT - 1:
                                # normalize this q stripe as soon as its
                                # accumulation closes:
                                #   attout^T = O'[0:DH] * recip(O'[DH])
                                # (att_scale is folded into the v_aug ones
                                # column).  reciprocal_approx_fast silently
                                # corrupts on HW when its input AP starts at
                                # partition 64, so stage the denominator row
                                # at partition 0 first (row 0 of the bc
                                # tile, which the broadcast then overwrites
                                # -- its RAW dep on r guarantees the recip
                                # consumed it first).
                                if qi == 0:
                                    r_e = rbuf.tile([1, TOK], F32, tag="R")
                                    r_o = rbuf.tile([1, TOK], F32, tag="R")
                                    bc_e = bcp.tile([DH, TOK], F32, tag="BC")
                                    bc_o = bcp.tile([DH, TOK], F32, tag="BC")
                                    nrm[0] = (ps_oe, r_e, bc_e)
                                    nrm[1] = (ps_oo, r_o, bc_o)
                                for ri in (0, 1):
                                    ps_o, r_sb, bc_sb = nrm[ri]
                                    nc.vector.tensor_copy(
                                        out=bc_sb[0:1, q0 : q0 + qw],
                                        in_=ps_o[DH : DH + 1, q0 : q0 + qw],
                                    )
                                    nc.vector.reciprocal_approx_fast(
                                        out=r_sb[0:1, q0 : q0 + qw],
                                        in_=bc_sb[0:1, q0 : q0 + qw],
                                    )
                                    nc.gpsimd.partition_broadcast(
                                        bc_sb[:, q0 : q0 + qw],
                                        r_sb[0:1, q0 : q0 + qw],
                                        channels=DH,
                                    )
                                    nc.vector.tensor_mul(
                                        out=ao_sb[
                                            ri * DH : (ri + 1) * DH,
                                            p,
                                            q0 : q0 + qw,
                                        ],
                                        in0=ps_o[0:DH, q0 : q0 + qw],
                                        in1=bc_sb[:, q0 : q0 + qw],
                                    )
                            if enext is not None:
                                ecur[qi] = enext

                for p in range(NPAIR - 1):
                    attn_pair(p)

            # x freed: the wp prefetch DMAs (arena-aliased with x) run
            # during the last pair's attention, and the last pair's slots
            # (which have no Q/K work left) are filled with the first
            # projection groups' partial contractions (kt <= KT-2; the
            # pair's own ao only enters at kt = KT-1).  Projection PSUM
            # comes from psQK -- free once the Q/K stream is exhausted --
            # so no proj matmul ever waits on the attention accumulators.
            with tc.tile_pool(name="wpp", bufs=2) as wpp:
                w_dp = w_d["wp"][:, :].rearrange("p (kt n) -> p kt n", n=D)
                pj_groups = []
                for c0, cw in _chunks(D, 512):
                    wp_sb = wpp.tile([P, KT, 512], BF16, tag="wp")
                    nc.sync.dma_start(
                        out=wp_sb[:, :, 0:cw], in_=w_dp[:, :, c0 : c0 + cw]
                    )
                    for mt in range(MT):
                        pj_groups.append((c0, cw, wp_sb, mt))

                pj = {"gi": 0, "kt": 0, "ps": None}

                def pj_emit(n, during_attn=False):
                    for _ in range(n):
                        gi = pj["gi"]
                        if gi >= len(pj_groups):
                            return
                        if during_attn and (gi >= 2 or pj["kt"] >= KT - 1):
                            return
                        c0, cw, wp_sb, mt = pj_groups[gi]
                        kt = pj["kt"]
                        if kt == 0:
                            ps_p = psQK.tile([P, 512], F32, tag="psQK")
                            pj["ps"] = ps_p
                        nc.tensor.matmul(
                            pj["ps"][:, 0:cw],
                            lhsT=ao_sb[:, kt, mt * P : (mt + 1) * P],
                            rhs=wp_sb[:, kt, 0:cw],
                            start=(kt == 0),
                            stop=(kt == KT - 1),
                        )
                        if kt == KT - 1:
                            o_sb = outp.tile([P, 512], F32, tag="o")
                            nc.vector.tensor_add(
                                out=o_sb[:, 0:cw],
                                in0=pj["ps"][:, 0:cw],
                                in1=bp_bc[:, c0 : c0 + cw],
                            )
                            nc.sync.dma_start(
                                out=out_d[mt * P : (mt + 1) * P, c0 : c0 + cw],
                                in_=o_sb[:, 0:cw],
                            )
                            pj["kt"] = 0
                            pj["gi"] += 1
                        else:
                            pj["kt"] = kt + 1

                attn_pair(NPAIR - 1, filler=lambda n: pj_emit(n, True))
                pj_emit(len(pj_groups) * (KT + 1))

    return nc


# ---------------------------------------------------------------------------
# host-side layout prep
# ---------------------------------------------------------------------------

def _round_f32r(x):
    """RNE to f32r's 11-explicit-mantissa-bit grid (matches HW rounding)."""
    u = np.ascontiguousarray(x, np.float32).view(np.uint32)
    u = ((u + np.uint32(1 << 11)) >> 12) << 12
    return u.view(np.float32)


def host_prep_shared(w_qkv, b_qkv, w_proj, b_proj, D, H):
    """Split/retile the weights once for all cores."""
    KT = D // P
    NPAIR = H // 2

    def tile_w(w):  # [D, N] -> [P, KT*N]
        N = w.shape[1]
        return _round_f32r(
            w.reshape(KT, P, N).transpose(1, 0, 2).reshape(P, KT * N)
        )

    def tile_w_bf16(w):
        import ml_dtypes

        N = w.shape[1]
        return np.ascontiguousarray(
            w.reshape(KT, P, N).transpose(1, 0, 2).reshape(P, KT * N)
        ).astype(ml_dtypes.bfloat16)

    wq3 = w_qkv.reshape(D, H, DH, 3)
    out = {
        "wq": tile_w(np.ascontiguousarray(wq3[:, :, :, 0].reshape(D, D))),
        "wk": tile_w(np.ascontiguousarray(wq3[:, :, :, 1].reshape(D, D))),
        "wv": tile_w_bf16(np.ascontiguousarray(wq3[:, :, :, 2].reshape(D, D))),
        "wp": tile_w_bf16(np.ascontiguousarray(w_proj)),
    }
    b3 = b_qkv.reshape(H, DH, 3)
    bq = np.ascontiguousarray(b3[:, :, 0].reshape(D))
    bk = np.ascontiguousarray(b3[:, :, 1].reshape(D))
    bv = np.ascontiguousarray(b3[:, :, 2].reshape(D))
    out["bq"] = np.ascontiguousarray(bq.reshape(NPAIR, P).T).astype(np.float32)
    out["bk"] = np.ascontiguousarray(bk.reshape(NPAIR, P).T).astype(np.float32)
    import ml_dtypes

    out["bv"] = bv.reshape(1, D).astype(ml_dtypes.bfloat16)
    out["bp"] = np.asarray(b_proj, np.float32).reshape(1, D).astype(
        ml_dtypes.bfloat16
    )
    return out


def host_prep_x(x_b, TOK, D):
    """One batch element [TOK, D] -> x^T tiled [P, KT*TOK]."""
    KT = D // P
    xT = np.ascontiguousarray(np.asarray(x_b, np.float32).T)  # [D, TOK]
    return _round_f32r(
        xT.reshape(KT, P, TOK).transpose(1, 0, 2).reshape(P, KT * TOK)
    )


def host_prep_x_bf16(x_b, TOK, D):
    import ml_dtypes

    KT = D // P
    xT = np.ascontiguousarray(np.asarray(x_b, np.float32).T)
    return (
        xT.reshape(KT, P, TOK)
        .transpose(1, 0, 2)
        .reshape(P, KT * TOK)
        .astype(ml_dtypes.bfloat16)
    )


# ---------------------------------------------------------------------------
# entry point
# ---------------------------------------------------------------------------

_BUILT = {}


def _get_nc(TOK, D, H, att_scale):
    key = (TOK, D, H, att_scale)
    if key not in _BUILT:
        nc = bacc.Bacc(
            "TRN2",
            target_bir_lowering=False,
            debug=False,
            dynamic_dma_scratch_size=512,
        )
        build(nc, TOK, D, H, att_scale)
        nc.compile()
        nc.finalize()
        _BUILT[key] = nc
    return _BUILT[key]


def kernel(x, w_qkv, b_qkv, w_proj, b_proj):
    from concourse.bass_utils import run_bass_kernel_spmd

    x = np.asarray(x, np.float32)
    B, TOK, D = x.shape
    H = H_FULL
    shared = host_prep_shared(
        np.asarray(w_qkv, np.float32),
        np.asarray(b_qkv, np.float32),
        np.asarray(w_proj, np.float32),
        np.asarray(b_proj, np.float32),
        D,
        H,
    )
    in_maps = []
    for b in range(B):
        m = dict(shared)
        m["x"] = host_prep_x(x[b], TOK, D)
        m["xb"] = host_prep_x_bf16(x[b], TOK, D)
        in_maps.append(m)

    nc = _get_nc(TOK, D, H, ATT_SCALE_FULL)
    res = run_bass_kernel_spmd(nc, in_maps, list(range(N_CORES)))
    out = np.stack([res.results[b]["out"] for b in range(B)], axis=0)
    return out.astype(np.float32)

